# revision 2
# baseline (speedup 1.0000x reference)
"""MoE QLoRA linear kernel for Trainium2, hybrid-sharded 4 token groups x 2
feature halves (each of 8 cores: 2048 tokens x 2048 out-features).

Math per core (x fp16-cast, all matmuls fp16 with fp32 PSUM accumulation):
  phase 1: proj rows (x @ [A;Wr]^T, 36 rows) per 512-token slab; softmax via
           exp + tiny PE partition-reduction matmuls; wproj = SCALE*gate*proj
           written fp16 (rows 0..31; row 32 = ones for the bias fold).
  phase 2: per o-tile (128 out-feats): W-tile-stationary k-loop over 32
           k-tiles x 4 token slabs into PSUM, + 1 lora matmul (k=33, rows =
           [Bm rows; b]) accumulated into the same PSUM group; copy out
           (DVE/ACT split) and DMA (OUT_shard, tok) per o-tile.

vs pure token-DP (1024 tok x 4096 feats): halving the feature dim doubles the
tokens per core, so each stationary W-tile feeds 4 consecutive matmuls
instead of 2. The toolchain runs walrus with --enable-ldw-opt=false, so the
post-Tile BIR reloads the identical stationary before every matmul
(~25-70ns each); a post-compile pass (_dedupe_ldweights) removes the 1502
redundant consecutive reloads (2240 -> ~700 InstLdweights), which hardware
validation shows is bit-exact for THIS instruction stream. The cost is
phase-1 (router+proj) running twice per token group (+14us) -- net win.

PSUM budget (8 banks): phase-1 rotates proj slabs through 2 banks + 2 for
gating reductions; phase-2 rotates (o-tile, slab) tiles through 5 banks so
the copy-out drain of the previous o-tile never stalls the k-loop.

NOTE on surgery safety: deleting redundant LDWEIGHTS is validated on
hardware for this exact build (bit-identical output to the undeduped
kernel). A restructured variant (k-major phase-1 + per-slab output DMA)
produced deterministic corruption when its (different) duplicate set was
deleted, so do NOT reorder the emission without re-validating on hardware.
"""

import numpy as np

import concourse.bass as bass
import concourse.tile as tile
from concourse import bacc, mybir
from concourse import bass_utils

B, S, IN, OUT, E, R = 4, 2048, 4096, 4096, 4, 8
SCALE = 16.0 / 8.0
N_CORES = 8
TOK = B * S                  # 8192 tokens
TG = 4                       # token groups
FH = 2                       # feature halves
T2 = TOK // TG               # 2048 tokens per core
OH = OUT // FH               # 2048 out-features per core
P = 128
KT = IN // P                 # 32 k-tiles
OT2 = OH // P                # 16 o-tiles per core
NSLAB = 512
NS = T2 // NSLAB             # 4 token slabs
ER = E * R                   # 32
ERA = ER + 1

F16 = mybir.dt.float16
F32 = mybir.dt.float32

_NC = None

DIRECT_PSUM_DMA = False


def _dedupe_ldweights(nc):
    """Remove PE InstLdweights that reload the identical weights AP already
    in the array (no other Ldweights between); move their waits to the next
    PE instruction (in-order queue => equivalent). Keep any with updates."""

    def ldw_key(inst):
        ap = inst.ins[0]
        return (
            ap.memref,
            ap.offset,
            tuple(tuple(p) for p in ap.ap),
            str(ap.dtype),
            getattr(inst, "perf_mode", None),
            getattr(inst, "is_transpose", None),
            tuple(inst.tile_position) if getattr(inst, "tile_position", None) else None,
        )

    pe = mybir.EngineType.PE
    removed = 0
    for fn in nc.m.functions:
        for blk in fn.blocks:
            insts = list(blk.instructions)
            last_key = None
            keep = []
            pending = []
            changed = False
            for inst in insts:
                if getattr(inst, "engine", None) != pe:
                    keep.append(inst)
                    continue
                if isinstance(inst, mybir.InstLdweights):
                    key = ldw_key(inst)
                    si = inst.sync_info
                    if key == last_key and (si is None or len(si.on_update) == 0):
                        if si is not None and len(si.on_wait) > 0:
                            pending.extend(si.on_wait)
                        removed += 1
                        changed = True
                        continue
                    last_key = key
                if pending:
                    si = inst.sync_info
                    if si is None:
                        inst.sync_info = mybir.SyncInfo(
                            on_wait=list(pending), on_update=[]
                        )
                    else:
                        si.on_wait = list(si.on_wait) + list(pending)
                    pending = []
                keep.append(inst)
            assert not pending
            if changed:
                blk.instructions = keep
    return removed


def build_nc(reps=1, dedupe=True):
    nc = bacc.Bacc("TRN2", target_bir_lowering=False, debug=False)

    xd = nc.dram_tensor("xd", [P, KT, T2], F16, kind="ExternalInput")
    wd = nc.dram_tensor("wd", [OT2, P, KT, P], F16, kind="ExternalInput")
    artd = nc.dram_tensor("artd", [P, KT, ER + E], F16, kind="ExternalInput")
    btd = nc.dram_tensor("btd", [ERA, OH], F16, kind="ExternalInput")
    seld = nc.dram_tensor("seld", [E, ER], F32, kind="ExternalInput")
    od = nc.dram_tensor("od", [OH, T2], F32, kind="ExternalOutput")

    with tile.TileContext(nc) as tc:
        with (
            tc.tile_pool(name="consts", bufs=1) as consts,
            tc.tile_pool(name="wpool", bufs=3) as wpool,
            tc.tile_pool(name="opool", bufs=3) as opool,
            tc.tile_pool(name="small", bufs=2) as small,
            tc.tile_pool(name="psum_proj", bufs=2, space="PSUM") as psum_proj,
            tc.tile_pool(name="psum_base", bufs=1, space="PSUM") as psum_base,
        ):
            art_sb = consts.tile([P, KT, ER + E], F16)
            nc.sync.dma_start(out=art_sb[:], in_=artd[:])
            bt_sb = consts.tile([ERA, OH], F16)
            nc.sync.dma_start(out=bt_sb[:], in_=btd[:])
            sel_sb = consts.tile([E, ER], F32)
            nc.sync.dma_start(out=sel_sb[:], in_=seld[:])

            w_tiles = {}

            def load_w(ot):
                w_sb = wpool.tile([P, KT, P], F16, tag="w", name="w_sb")
                nc.sync.dma_start(out=w_sb[:], in_=wd[ot])
                w_tiles[ot] = w_sb

            load_w(0)
            load_w(1)

            # Resident activations: x^T tiled (p, k, t), fp16, 16 MiB.
            x_sb = consts.tile([P, KT, T2], F16)
            for k in range(KT):
                nc.sync.dma_start(out=x_sb[:, k, :], in_=xd[:, k, :])

            ones_e1 = consts.tile([E, 1], F32)
            nc.vector.memset(ones_e1[:], 1.0)
            ones_1e = consts.tile([1, E], F32)
            nc.vector.memset(ones_1e[:], 1.0)
            wp_sb = consts.tile([ERA, T2], F16)
            nc.vector.memset(wp_sb[ER : ER + 1, :], 1.0)

            # ---------- phase 1: proj+router, slab-pipelined ----------
            def proj_slab(t):
                tsl = slice(t * NSLAB, (t + 1) * NSLAB)
                pp = psum_proj.tile([ER + E, NSLAB], F32, tag="pp", name="pp")
                for k in range(KT):
                    nc.tensor.matmul(
                        pp[:],
                        art_sb[:, k, :],
                        x_sb[:, k, tsl],
                        start=(k == 0),
                        stop=(k == KT - 1),
                    )
                return pp

            def gating(t, pp):
                tsl = slice(t * NSLAB, (t + 1) * NSLAB)
                e_sb = small.tile([E, NSLAB], F32, tag="e", name="e_sb")
                nc.scalar.activation(
                    e_sb[:], pp[ER : ER + E, :], mybir.ActivationFunctionType.Exp
                )
                s_ps = psum_gat.tile([1, NSLAB], F32, tag="gat", name="s_ps")
                nc.tensor.matmul(s_ps[:], ones_e1[:], e_sb[:])
                r_sb = small.tile([1, NSLAB], F32, tag="r", name="r_sb")
                nc.vector.reciprocal(r_sb[:], s_ps[:])
                r4_ps = psum_proj.tile([E, NSLAB], F32, tag="gat", name="r4_ps")
                nc.tensor.matmul(r4_ps[:], ones_1e[:], r_sb[:])
                g4_sb = small.tile([E, NSLAB], F32, tag="g4", name="g4_sb")
                nc.vector.tensor_mul(g4_sb[:], e_sb[:], r4_ps[:])
                g32_ps = psum_proj.tile([ER, NSLAB], F32, tag="gat", name="g32_ps")
                nc.tensor.matmul(g32_ps[:], sel_sb[:], g4_sb[:])
                g32_sb = small.tile([ER, NSLAB], F32, tag="g32s", name="g32_sb")
                nc.vector.tensor_copy(g32_sb[:], g32_ps[:])
                nc.vector.tensor_mul(wp_sb[0:ER, tsl], pp[0:ER, :], g32_sb[:])

            pps = {}
            for t in range(NS):
                pps[t] = proj_slab(t)
                if t >= 1:
                    gating(t - 1, pps.pop(t - 1))
            gating(NS - 1, pps.pop(NS - 1))

            # ---------- phase 2 ----------
            def do_ot(ot, next_ot):
                if ot not in w_tiles:
                    load_w(ot)
                if next_ot is not None and next_ot not in w_tiles:
                    load_w(next_ot)
                osl = slice(ot * P, (ot + 1) * P)
                pots = [
                    psum_base.tile([P, NSLAB], F32, tag=f"po{t}", name=f"po{t}")
                    for t in range(NS)
                ]
                for k in range(KT):
                    for t in range(NS):
                        nc.tensor.matmul(
                            pots[t][:],
                            w_tiles[ot][:, k, :],
                            x_sb[:, k, t * NSLAB : (t + 1) * NSLAB],
                            start=(k == 0),
                            stop=False,
                        )
                for t in range(NS):
                    nc.tensor.matmul(
                        pots[t][:],
                        bt_sb[:, osl],
                        wp_sb[:, t * NSLAB : (t + 1) * NSLAB],
                        start=False,
                        stop=True,
                    )
                if DIRECT_PSUM_DMA:
                    for t in range(NS):
                        nc.sync.dma_start(
                            out=od[osl, t * NSLAB : (t + 1) * NSLAB], in_=pots[t][:]
                        )
                else:
                    o_sb = opool.tile([P, T2], F32, tag="o", name="o_sb")
                    for t in range(NS):
                        cp = nc.vector.tensor_copy if t % 2 == 0 else nc.scalar.copy
                        cp(o_sb[:, t * NSLAB : (t + 1) * NSLAB], pots[t][:])
                    nc.sync.dma_start(out=od[osl, :], in_=o_sb[:])
                del w_tiles[ot]

            for rep in range(reps):
                for ot in range(OT2):
                    do_ot(ot, ot + 1 if ot + 1 < OT2 else None)

    nc.compile()
    if dedupe:
        n = _dedupe_ldweights(nc)
        assert n > 0
    return nc


def get_nc():
    global _NC
    if _NC is None:
        _NC = build_nc()
    return _NC


def _prep_shared(W, b, A, Bm, Wr):
    # W halves: wdh[h][ot, p, k, o] = W[h*OH + ot*128 + o, k*128 + p]
    wdh = []
    for h in range(FH):
        Wh = W[h * OH : (h + 1) * OH]
        wdh.append(
            np.ascontiguousarray(
                Wh.reshape(OT2, P, KT, P).transpose(0, 3, 2, 1).astype(np.float16)
            )
        )
    ar = np.concatenate([A.reshape(ER, IN), Wr], axis=0)
    artd = np.ascontiguousarray(
        ar.T.reshape(KT, P, ER + E).transpose(1, 0, 2).astype(np.float16)
    )
    bt = np.concatenate([Bm.transpose(0, 2, 1).reshape(ER, OUT), b[None, :]], axis=0)
    btdh = [
        np.ascontiguousarray(bt[:, h * OH : (h + 1) * OH].astype(np.float16))
        for h in range(FH)
    ]
    sel = np.zeros((E, ER), np.float32)
    for e in range(E):
        sel[e, e * R : (e + 1) * R] = SCALE
    return wdh, artd, btdh, sel


def make_in_maps(x, W, b, A, Bm, Wr):
    xt = np.asarray(x, np.float32).reshape(TOK, IN)
    wdh, artd, btdh, sel = _prep_shared(
        np.asarray(W, np.float32),
        np.asarray(b, np.float32),
        np.asarray(A, np.float32),
        np.asarray(Bm, np.float32),
        np.asarray(Wr, np.float32),
    )
    xg = []
    for g in range(TG):
        xs = xt[g * T2 : (g + 1) * T2]
        xg.append(
            np.ascontiguousarray(
                xs.T.reshape(KT, P, T2).transpose(1, 0, 2).astype(np.float16)
            )
        )
    maps = []
    for c in range(N_CORES):
        g, h = c // FH, c % FH
        maps.append(
            {"xd": xg[g], "wd": wdh[h], "artd": artd, "btd": btdh[h], "seld": sel}
        )
    return maps


def gather_out(results):
    out = np.empty((TOK, OUT), np.float32)
    for c, r in enumerate(results):
        g, h = c // FH, c % FH
        out[g * T2 : (g + 1) * T2, h * OH : (h + 1) * OH] = r["od"].T
    return out.reshape(B, S, OUT)


def kernel(x, W, b, A, Bm, Wr, _trace=False):
    nc = get_nc()
    in_maps = make_in_maps(x, W, b, A, Bm, Wr)
    res = bass_utils.run_bass_kernel_spmd(
        nc, in_maps, core_ids=list(range(N_CORES)), trace=_trace
    )
    out = gather_out(res.results)
    if _trace:
        return out, res
    return out
